# revision 6
# baseline (speedup 1.0000x reference)
"""Trainium2 Bass kernel for the OFPenalty eigenvalue-penalty loss.

Math (per sample b of 256):
  W = x[b] reshaped [C=2048, N=49];  G = W^T W  (49x49 Gram matrix)
  run1: x9 = G^9 x0 (normalization deferred - scale invariant),
        largest = Rayleigh(G, x9) = (G x9 . x9) / (x9 . x9)
  run2: B = G - largest*I, u9 = B^9 x1 (x1 = scaled x9),
        tmp = Rayleigh(B, u9); smallest = tmp + largest
  penalty = (largest/smallest - 1)^2 = (tmp/smallest)^2; output = mean.

Implementation notes:
  - Pure data parallel: 32 samples per core on 8 cores.  Samples are
    processed in pairs packed along partitions: sample 2p in rows 0:64
    (real 0:49), sample 2p+1 in rows 64:128 (real 64:113).  All square
    matrices use a PACKED [128, 64] layout (block b occupies rows
    64b:64b+49, cols 0:49 of its half) so every PSUM->SBUF move is one
    [128, 64] copy.
  - Gram matmuls run in bf16 (1 PE cycle/row vs 4 for fp32); x tiles are
    converted fp32->bf16 once on ACT/Pool.  Stationary and moving tiles
    are zero-padded to 64 columns so all pad rows/cols of every product
    stay exactly 0 - no masks needed anywhere downstream.
  - G^9 x0 is evaluated log-depth: G2=G^2, G4=G2^2, G8=G4^2 (bf16
    squarings, fp32 PSUM accumulate), then x9 = G8*(G*x0), w1 = A*x9.
    Run2: Ball = A - lam*I (built on DVE from Abf + rank-1 broadcast
    lamI), B2/B4/B8 squarings, u1 = S102*(w1 - lam*x9) directly from
    run1 PSUMs (no extra matvec), u9 = B8*u1, w2 = Ball*u9.
  - Rayleigh quotients: per-sample columns T = (v*S104)*w on DVE read
    the matvec PSUMs directly, a ones-vector matmul reduces over
    partitions, scalar chain on free-dim rows.  Scalings by powers of
    two (exact) keep all intermediates inside fp32 range.
  - Pairs stream one at a time behind the (serialized, 360 B/ns) DMA
    queue; Rayleigh/shift work is batched in groups [6,6,3,1] so only
    the last small group's chain is exposed after the final DMA.
"""

import os
import sys
from contextlib import ExitStack

import numpy as np

for _p in ("/opt/trn_rl_repo",):
    if os.path.isdir(_p) and _p not in sys.path:
        sys.path.insert(0, _p)

import concourse.bass as bass  # noqa: E402
import concourse.tile as tile  # noqa: E402
from concourse import bacc, mybir  # noqa: E402
from concourse.bass_utils import run_bass_kernel_spmd  # noqa: E402

F32 = mybir.dt.float32
BF16 = mybir.dt.bfloat16
ALU = mybir.AluOpType

B, C, N = 256, 2048, 49
NCORES = 8
BS = B // NCORES  # 32 samples per core
NPAIR = BS // 2  # 16 pairs
KT = C // 128  # 16 contraction tiles
PG = 128
B1 = 64  # partition base of the second sample in a pair
PD = 64  # padded block width (cols 49:64 always zero)
S52 = float(2.0**-52)
S104 = float(2.0**-104)  # scale one side of Rayleigh products
S102 = float(2.0**-102)  # rescale x9 -> x1 (run2 warm start)
GROUPS = [6, 6, 3, 1]  # pairs per Rayleigh group (sum = NPAIR)
GWMAX = max(GROUPS)


def _emit(tc, x, x0, pen, repeat=1):
    nc = tc.nc
    ctx = ExitStack()
    with ctx:
        const = ctx.enter_context(tc.tile_pool(name="const", bufs=1))
        xpool = ctx.enter_context(tc.tile_pool(name="xt", bufs=3))
        ps_gram = ctx.enter_context(tc.tile_pool(name="ps_gram", bufs=2, space="PSUM"))
        ps_sq = ctx.enter_context(tc.tile_pool(name="ps_sq", bufs=2, space="PSUM"))
        ps_mv = ctx.enter_context(tc.tile_pool(name="ps_mv", bufs=2, space="PSUM"))
        ps_nd = ctx.enter_context(tc.tile_pool(name="ps_nd", bufs=1, space="PSUM"))
        ps_psl = ctx.enter_context(tc.tile_pool(name="ps_psl", bufs=1, space="PSUM"))

        # ---- constants -------------------------------------------------
        # x0 columns: X0[0:49, p] = x0[2p], X0[64:113, p] = x0[2p+1]
        X0 = const.tile([PG, NPAIR], BF16)
        X0F = const.tile([PG, NPAIR], F32)
        nc.gpsimd.memset(X0F[:], 0.0)
        x0r = x0.rearrange("(p two) j -> two j p", two=2)
        nc.sync.dma_start(X0F[0:N, :], x0r[0])
        nc.sync.dma_start(X0F[B1 : B1 + N, :], x0r[1])
        nc.vector.tensor_copy(X0[:], X0F[:])

        # packed identity: DIAG[q, j] = 1 iff (q % 64) == j
        DIAG = const.tile([PG, PD], F32)
        nc.gpsimd.memset(DIAG[:], 0.0)
        for blk in range(2):
            nc.gpsimd.affine_select(
                out=DIAG[blk * B1 : (blk + 1) * B1, :],
                in_=DIAG[blk * B1 : (blk + 1) * B1, :],
                compare_op=ALU.not_equal,
                fill=1.0,
                base=0,
                pattern=[[-1, PD]],
                channel_multiplier=1,
            )

        # block-ownership row masks for the rank-1 lambda broadcast
        CM0 = const.tile([1, PG], F32)
        nc.gpsimd.memset(CM0[:], 0.0)
        nc.gpsimd.memset(CM0[:, 0:N], 1.0)
        CM1 = const.tile([1, PG], F32)
        nc.gpsimd.memset(CM1[:], 0.0)
        nc.gpsimd.memset(CM1[:, B1 : B1 + N], 1.0)

        ONE128 = const.tile([PG, 1], F32)
        nc.gpsimd.memset(ONE128[:], 1.0)

        # ---- persistent state ------------------------------------------
        NXB = 3  # bf16 x-tile slots (pads memset once)
        xb = []
        for i in range(NXB):
            t = const.tile([PG, 2, KT, PD], BF16, name=f"xb{i}")
            nc.gpsimd.memset(t[:, :, :, N:PD], 0.0)
            xb.append(t)

        NA = 8  # Abf lives until its group's Ball build
        Abf = [const.tile([PG, PD], BF16, name=f"Abf{i}") for i in range(NA)]
        NSQ = 3
        G2bf = [const.tile([PG, PD], BF16, name=f"G2bf{i}") for i in range(NSQ)]
        G4bf = [const.tile([PG, PD], BF16, name=f"G4bf{i}") for i in range(NSQ)]
        G8bf = [const.tile([PG, PD], BF16, name=f"G8bf{i}") for i in range(NSQ)]
        NB = 2
        Ballbf = [const.tile([PG, PD], BF16, name=f"Ballbf{i}") for i in range(NB)]
        B2bf = [const.tile([PG, PD], BF16, name=f"B2bf{i}") for i in range(NB)]
        B4bf = [const.tile([PG, PD], BF16, name=f"B4bf{i}") for i in range(NB)]
        B8bf = [const.tile([PG, PD], BF16, name=f"B8bf{i}") for i in range(NB)]
        SCR = [const.tile([PG, PD + 4], F32, name=f"SCR{i}") for i in range(NB)]

        Y = const.tile([PG, NPAIR], BF16)  # y = A*x0 columns
        X9 = const.tile([PG, NPAIR], BF16)  # x9 columns (bf16 for matvec)
        U1 = const.tile([PG, NPAIR], BF16)  # run2 warm starts
        U9 = const.tile([PG, NPAIR], BF16)
        LAMV = const.tile([PG, GWMAX], F32)  # per-partition lambda (per group)
        T1 = const.tile([PG, 4 * NPAIR], F32)  # Rayleigh-1 product columns
        T2 = const.tile([PG, 4 * NPAIR], F32)  # Rayleigh-2 product columns
        nc.gpsimd.memset(T1[:], 0.0)
        nc.gpsimd.memset(T2[:], 0.0)
        LAM = const.tile([1, BS], F32)  # lambda rows, [s0 cols | s1 cols] per grp
        RD = const.tile([1, BS], F32)
        TMPr = const.tile([1, BS], F32)
        SM = const.tile([1, BS], F32)
        RS = const.tile([1, BS], F32)
        RT = const.tile([1, BS], F32)
        PEN = const.tile([1, BS], F32)

        for _rep in range(repeat):
            # DMA layout: partition q holds c-rows {512b + 4q + r : r<4};
            # 784B-contiguous descriptors (>=512B keeps DMA at full rate).
            xrs = x.rearrange(
                "(p two) (b q r) j -> p two q b (r j)", two=2, b=4, q=128, r=4
            )

            gram_ps = {}  # pair -> psum tile (held until group Rayleigh/u1)
            mv1_ps = {}  # group -> [PG, 2*gw] psum (x9 | w1)
            mv2_ps = {}  # group -> [PG, 2*gw] psum (u9 | w2)

            pair_group = []
            for g, gw in enumerate(GROUPS):
                pair_group += [g] * gw
            group_base = [sum(GROUPS[:g]) for g in range(len(GROUPS))]

            def matvec(ps_out, col, stat, mov_col, accum=False):
                # block-diagonal matvec in the packed layout: two [64,64]
                # stationary quadrants
                for blk in range(2):
                    r0 = blk * B1
                    nc.tensor.matmul(
                        ps_out[r0 : r0 + B1, col : col + 1],
                        stat[r0 : r0 + B1, :],
                        mov_col[r0 : r0 + B1, :],
                        start=not accum,
                        stop=True,
                    )

            def square(dst_bf, src_bf, copy_eng, name):
                ps = ps_sq.tile([PG, PD], F32, tag="sq", name=name)
                for blk in range(2):
                    r0 = blk * B1
                    nc.tensor.matmul(
                        ps[r0 : r0 + B1, :],
                        src_bf[r0 : r0 + B1, :],
                        src_bf[r0 : r0 + B1, :],
                        start=True,
                        stop=True,
                    )
                if copy_eng is nc.scalar:
                    copy_eng.copy(dst_bf[:], ps[:])
                else:
                    copy_eng.tensor_copy(dst_bf[:], ps[:])
                return ps

            for p in range(NPAIR):
                g = pair_group[p]
                gw = GROUPS[g]
                gb = group_base[g]
                pc = p - gb  # column within group

                # ---- load + convert + Gram ------------------------------
                xt = xpool.tile([128, 2, KT * N], F32, tag="xt", name=f"xt{p}")
                for s in range(2):
                    nc.sync.dma_start(
                        xt[:, s, :].rearrange("q (b m) -> q b m", b=4),
                        xrs[p, s],
                    )
                xbt = xb[p % NXB]
                xsrc = xt.rearrange("q s (b r j) -> q s (b r) j", r=4, j=N)
                nc.gpsimd.tensor_copy(xbt[:, 0, :, 0:N], xsrc[:, 0])
                nc.scalar.copy(xbt[:, 1, :, 0:N], xsrc[:, 1])

                ps = ps_gram.tile([PG, PD], F32, tag="gram", name=f"gram{p}")
                for k in range(KT):
                    for s in range(2):
                        r0 = s * B1
                        wk = xbt[:, s, k, :]
                        nc.tensor.matmul(
                            ps[r0 : r0 + B1, :],
                            wk,
                            wk,
                            start=(k == 0),
                            stop=(k == KT - 1),
                        )
                gram_ps[p] = ps
                A = Abf[p % NA]
                nc.scalar.copy(A[:], ps[:])

                # ---- run1: y, squarings, x9, w1 -------------------------
                if pc == 0:
                    mv1_ps[g] = ps_mv.tile(
                        [PG, 2 * GWMAX], F32, tag="mv", name=f"mv1_{g}"
                    )
                yps = ps_sq.tile([PG, PD], F32, tag="sq", name=f"y{p}")
                matvec(yps, 0, A, X0[:, p : p + 1])
                nc.scalar.copy(Y[:, p : p + 1], yps[:, 0:1])

                g2 = G2bf[p % NSQ]
                g4 = G4bf[p % NSQ]
                g8 = G8bf[p % NSQ]
                square(g2, A, nc.gpsimd, f"g2_{p}")
                square(g4, g2, nc.vector, f"g4_{p}")
                square(g8, g4, nc.scalar, f"g8_{p}")

                m1 = mv1_ps[g]
                matvec(m1, pc, g8, Y[:, p : p + 1])
                nc.scalar.copy(X9[:, p : p + 1], m1[:, pc : pc + 1])
                matvec(m1, gw + pc, A, X9[:, p : p + 1])

                # Rayleigh-1 product columns (read matvec PSUM directly):
                # T1 cols [gb..]: num_s0 | num_s1 | den_s0 | den_s1 interleaved
                # per group: num block rows only, pads are zero.
                x9c = m1[:, pc : pc + 1]
                w1c = m1[:, gw + pc : gw + pc + 1]
                c0 = 4 * gb
                for blk in range(2):
                    r0 = blk * B1
                    tw = SCR[0][r0 : r0 + B1, 0:1]
                    tx = SCR[0][r0 : r0 + B1, 1:2]
                    nc.vector.tensor_scalar(
                        tw, w1c[r0 : r0 + B1, :], S104, None, op0=ALU.mult
                    )
                    nc.vector.tensor_scalar(
                        tx, x9c[r0 : r0 + B1, :], S104, None, op0=ALU.mult
                    )
                    ncol = c0 + blk * gw + pc
                    dcol = c0 + 2 * gw + blk * gw + pc
                    nc.vector.tensor_mul(
                        T1[r0 : r0 + B1, ncol : ncol + 1], tw, x9c[r0 : r0 + B1, :]
                    )
                    nc.vector.tensor_mul(
                        T1[r0 : r0 + B1, dcol : dcol + 1], tx, x9c[r0 : r0 + B1, :]
                    )

                # ---- group boundary: Rayleigh 1 + run2 ------------------
                if pc == gw - 1:
                    c0 = 4 * gb
                    nd = ps_nd.tile([1, 4 * GWMAX], F32, tag="nd", name=f"nd1_{g}")
                    nc.tensor.matmul(
                        nd[:, 0 : 4 * gw],
                        ONE128[:],
                        T1[:, c0 : c0 + 4 * gw],
                        start=True,
                        stop=True,
                    )
                    lam = LAM[:, 2 * gb : 2 * gb + 2 * gw]
                    rd = RD[:, 2 * gb : 2 * gb + 2 * gw]
                    nc.vector.reciprocal(rd, nd[:, 2 * gw : 4 * gw])
                    nc.vector.tensor_mul(lam, nd[:, 0 : 2 * gw], rd)

                    # LAMV[q, j] = lambda of the sample owning partition q
                    psl = ps_psl.tile([PG, GWMAX], F32, tag="psl", name=f"psl{g}")
                    nc.tensor.matmul(
                        psl[:, 0:gw], CM0[:], lam[:, 0:gw], start=True, stop=False
                    )
                    nc.tensor.matmul(
                        psl[:, 0:gw], CM1[:], lam[:, gw : 2 * gw],
                        start=False, stop=True,
                    )
                    nc.vector.tensor_copy(LAMV[:, 0:gw], psl[:, 0:gw])

                    mv2_ps[g] = ps_mv.tile(
                        [PG, 2 * GWMAX], F32, tag="mv", name=f"mv2_{g}"
                    )
                    m2 = mv2_ps[g]
                    for j in range(gw):
                        pj = gb + j
                        bb = Ballbf[pj % NB]
                        scr = SCR[pj % NB]
                        m1g = mv1_ps[g]
                        # u1 = S102*(w1 - lam*x9), from run1 PSUM columns
                        x9c = m1g[:, j : j + 1]
                        w1c = m1g[:, gw + j : gw + j + 1]
                        t = scr[:, 2:3]
                        nc.vector.tensor_mul(t, LAMV[:, j : j + 1], x9c)
                        d = scr[:, 3:4]
                        nc.vector.tensor_sub(d, w1c, t)
                        nc.vector.tensor_scalar(
                            U1[:, pj : pj + 1], d, S102, None, op0=ALU.mult
                        )
                        # Ball = A - lam*I (packed), built in bf16
                        lamI = scr[:, 4 : 4 + PD]
                        nc.vector.tensor_tensor(
                            lamI,
                            DIAG[:],
                            LAMV[:, j : j + 1].broadcast_to([PG, PD]),
                            op=ALU.mult,
                        )
                        nc.vector.tensor_sub(bb[:], Abf[pj % NA][:], lamI)

                        b2 = B2bf[pj % NB]
                        b4 = B4bf[pj % NB]
                        b8 = B8bf[pj % NB]
                        square(b2, bb, nc.gpsimd, f"b2_{pj}")
                        square(b4, b2, nc.vector, f"b4_{pj}")
                        square(b8, b4, nc.scalar, f"b8_{pj}")
                        matvec(m2, j, b8, U1[:, pj : pj + 1])
                        nc.scalar.copy(U9[:, pj : pj + 1], m2[:, j : j + 1])
                        matvec(m2, gw + j, bb, U9[:, pj : pj + 1])

                        u9c = m2[:, j : j + 1]
                        w2c = m2[:, gw + j : gw + j + 1]
                        c0b = 4 * gb
                        for blk in range(2):
                            r0 = blk * B1
                            tw = scr[r0 : r0 + B1, 0:1]
                            tx = scr[r0 : r0 + B1, 1:2]
                            nc.vector.tensor_scalar(
                                tw, w2c[r0 : r0 + B1, :], S104, None, op0=ALU.mult
                            )
                            nc.vector.tensor_scalar(
                                tx, u9c[r0 : r0 + B1, :], S104, None, op0=ALU.mult
                            )
                            ncol = c0b + blk * gw + j
                            dcol = c0b + 2 * gw + blk * gw + j
                            nc.vector.tensor_mul(
                                T2[r0 : r0 + B1, ncol : ncol + 1],
                                tw,
                                u9c[r0 : r0 + B1, :],
                            )
                            nc.vector.tensor_mul(
                                T2[r0 : r0 + B1, dcol : dcol + 1],
                                tx,
                                u9c[r0 : r0 + B1, :],
                            )

                    # ---- Rayleigh 2 + penalty for the group -------------
                    nd2 = ps_nd.tile([1, 4 * GWMAX], F32, tag="nd", name=f"nd2_{g}")
                    nc.tensor.matmul(
                        nd2[:, 0 : 4 * gw],
                        ONE128[:],
                        T2[:, c0 : c0 + 4 * gw],
                        start=True,
                        stop=True,
                    )
                    s = slice(2 * gb, 2 * gb + 2 * gw)
                    nc.vector.reciprocal(RD[:, s], nd2[:, 2 * gw : 4 * gw])
                    nc.vector.tensor_mul(TMPr[:, s], nd2[:, 0 : 2 * gw], RD[:, s])
                    nc.vector.tensor_add(SM[:, s], TMPr[:, s], LAM[:, s])
                    nc.vector.reciprocal(RS[:, s], SM[:, s])
                    nc.vector.tensor_mul(RT[:, s], TMPr[:, s], RS[:, s])
                    nc.vector.tensor_mul(PEN[:, s], RT[:, s], RT[:, s])

            nc.sync.dma_start(pen, PEN[:])


_NC_CACHE = {}


def build_nc(repeat=1):
    if repeat in _NC_CACHE:
        return _NC_CACHE[repeat]
    nc = bacc.Bacc("TRN2", target_bir_lowering=False, debug=False)
    x = nc.dram_tensor("x", [BS, C, N], F32, kind="ExternalInput")
    x0 = nc.dram_tensor("x0", [BS, N], F32, kind="ExternalInput")
    pen = nc.dram_tensor("pen", [BS], F32, kind="ExternalOutput")
    with tile.TileContext(nc) as tc:
        _emit(tc, x.ap(), x0.ap(), pen.ap(), repeat=repeat)
    nc.compile()
    _NC_CACHE[repeat] = nc
    return nc


LAST_RESULTS = None


def _unpermute(pens_core):
    """Device PEN layout -> per-sample order.

    Group g (pairs gb..gb+gw-1) owns columns [2*gb, 2*gb+2*gw): first gw
    cols are sample-0 (even) of each pair, next gw cols sample-1 (odd).
    """
    out = np.empty(BS, dtype=np.float64)
    gb = 0
    for gw in GROUPS:
        cols = pens_core[2 * gb : 2 * gb + 2 * gw]
        for j in range(gw):
            out[2 * (gb + j)] = cols[j]
            out[2 * (gb + j) + 1] = cols[gw + j]
        gb += gw
    return out


def kernel(x, x0):
    global LAST_RESULTS
    x = np.ascontiguousarray(np.asarray(x, dtype=np.float32).reshape(B, C, N))
    x0 = np.ascontiguousarray(np.asarray(x0, dtype=np.float32).reshape(B, N))
    nc = build_nc()
    in_maps = [
        {"x": x[i * BS : (i + 1) * BS], "x0": x0[i * BS : (i + 1) * BS]}
        for i in range(NCORES)
    ]
    trace = bool(int(os.environ.get("KERNEL_TRACE", "0")))
    res = run_bass_kernel_spmd(nc, in_maps, list(range(NCORES)), trace=trace)
    LAST_RESULTS = res
    total = 0.0
    for r in res.results:
        total += _unpermute(r["pen"].reshape(-1)).sum()
    return np.float32(total / B)
